# revision 1
# baseline (speedup 1.0000x reference)
"""Bass/Trainium2 kernel for nn_GNNPolicy_MILP (gnn_message_passing).

Strategy (8 NeuronCores, SPMD):
  - Host: cheap scalar graph prep on the nnz/constraint axis (segment sums via
    bincount, ~1.5% of total FLOPs), producing per-node z-inputs inv_s_v, x0,
    s_v. Nodes padded 100000 -> 100352 = 8*12544 and row-sharded per core.
  - Device (per core, fp32): two 128-wide embedding MLPs over 12544 nodes
    (feature-major layout, nodes in the matmul free dim), two conv updates
    with a [128] AllReduce each (global weighted node reduction), 3-layer
    output head. All dense FLOPs (~61 GFLOP total) run on the PE array.

Key algebraic reductions vs the reference (exact, not approximations):
  - emb_rhs is dead code; s_c/scaled_coef/s_v are identical across both convs.
  - mean(x_const) == (1/E) * sum_n s_v[n] * x_var[n]  -- the [50k,128]
    gather/scatter collapses to a weighted reduction over nodes.
"""
import numpy as np

import concourse.bass as bass
from concourse import bacc
import concourse.mybir as mybir
import concourse.tile as tile
from concourse.bass_utils import run_bass_kernel_spmd

NUM_NODES = 100000
NUM_EDGES = 50000
DEG = 16
HID = 128
NCORES = 8
NSH = 12544            # padded nodes per core (8*12544 = 100352)
NT = NSH // 128        # 98 rows of the [98,128] z layout
F32 = mybir.dt.float32

_CACHE = {}

_WLIST = [("pc2", 1, 96), ("b96", 96, 1), ("nw", 96, 64), ("nb", 64, 1),
          ("mw1", 64, 256), ("mb1", 128, 2), ("mw2", 128, 512), ("mb2", 128, 2),
          ("mw3", 128, 256), ("mb3", 128, 1), ("linw", 128, 256), ("linb", 128, 2),
          ("actw", 128, 256), ("actb", 128, 2), ("ow1", 128, 128), ("ob1", 128, 1),
          ("ow2", 128, 128), ("ob2", 128, 1), ("ow3", 128, 1), ("ob3", 1, 1),
          ("sig", 1, 2)]
WSPEC = {}
_o = 0
for _n, _r, _c in _WLIST:
    WSPEC[_n] = (_r, _c, _o)
    _o += _c
WCOLS = _o


# --------------------------------------------------------------------- host
def _host_prep(hyperedge_index, coef, rhs):
    row = np.asarray(hyperedge_index[0]).astype(np.int64)
    coef = np.asarray(coef, np.float32)
    rhs = np.asarray(rhs, np.float32).reshape(-1)

    cmat = coef.reshape(NUM_EDGES, DEG)
    s_c = np.abs(cmat).sum(1, dtype=np.float32)
    inv_s_c = np.where(s_c == 0, np.float32(0), np.float32(1) / s_c).astype(np.float32)
    sc = cmat * inv_s_c[:, None]
    rhs1 = rhs * inv_s_c
    rhs2 = rhs1 * inv_s_c
    sig1 = np.float32(rhs1.sum(dtype=np.float64))
    sig2 = np.float32(rhs2.sum(dtype=np.float64))

    s_v = np.bincount(row, weights=sc.ravel(), minlength=NUM_NODES).astype(np.float32)
    x0pre = np.bincount(row, weights=(sc * rhs1[:, None]).ravel(),
                        minlength=NUM_NODES).astype(np.float32)
    with np.errstate(divide="ignore"):
        inv_s_v = np.where(s_v == 0, np.float32(0),
                           np.float32(1) / s_v).astype(np.float32)
    x0 = (inv_s_v * x0pre).astype(np.float32)

    def shard(a):
        p = np.zeros(NCORES * NSH, np.float32)
        p[:NUM_NODES] = a
        return p.reshape(NCORES, NT, 128)

    return shard(inv_s_v), shard(x0), shard(s_v), sig1, sig2


# ------------------------------------------------------------------- device
def _build_nc():
    nc = bacc.Bacc(None, num_devices=NCORES)

    def inp(name, shape):
        return nc.dram_tensor(name, shape, F32, kind="ExternalInput")

    zinv_d = inp("zinv", [1, NSH])
    zx0_d = inp("zx0", [1, NSH])
    zsv_d = inp("zsv", [1, NSH])
    wpk_d = inp("wpk", [128, WCOLS])
    out_d = nc.dram_tensor("out", [1, NSH], F32, kind="ExternalOutput")

    AF = mybir.ActivationFunctionType
    ALU = mybir.AluOpType
    RG = [list(range(NCORES))]

    with tile.TileContext(nc) as tc:
        with (
            tc.tile_pool(name="persist", bufs=1) as pp,
            tc.tile_pool(name="work", bufs=2) as wp,
            tc.tile_pool(name="psum", bufs=6, space="PSUM") as pq,
            tc.tile_pool(name="dram", bufs=1, space="DRAM") as dp,
        ):
            # ---- one packed weight DMA
            wpk = pp.tile([128, WCOLS], F32, tag="wpk")
            nc.sync.dma_start(out=wpk[:], in_=wpk_d[:])

            def wsl(name):
                r, c, o = WSPEC[name]
                return wpk[0:r, o:o + c]

            sig = wsl("sig"); pc2 = wsl("pc2"); b96 = wsl("b96")
            nw = wsl("nw"); nb = wsl("nb")
            mw1 = wsl("mw1"); mb1 = wsl("mb1")
            mw2 = wsl("mw2"); mb2 = wsl("mb2")
            mw3 = wsl("mw3"); mb3 = wsl("mb3")
            linw = wsl("linw"); linb = wsl("linb")
            actw = wsl("actw"); actb = wsl("actb")
            ow1 = wsl("ow1"); ob1 = wsl("ob1")
            ow2 = wsl("ow2"); ob2 = wsl("ob2")
            ow3 = wsl("ow3"); ob3 = wsl("ob3")

            E = pp.tile([128, NSH], F32, tag="E")
            XV = pp.tile([128, NSH], F32, tag="XV")
            ones1 = pp.tile([1, 128], F32, tag="ones1")
            nc.vector.memset(ones1[:], 1.0)
            out_sb = pp.tile([1, NSH], F32, tag="osb")

            BLKS = [(b * 512, 512) for b in range(24)] + [(24 * 512, 256)]

            def emb_block(Zd, dst, n0, w):
                """dst[:, n0:n0+w] = emb(z) for nodes n0..n0+w, feature-major."""
                zr = wp.tile([1, 512], F32, tag="zr")
                nc.sync.dma_start(out=zr[:, :w], in_=Zd[0:1, n0:n0 + w])
                p_ps = pq.tile([96, 512], F32, tag="ps")
                nc.tensor.matmul(p_ps[:, :w], lhsT=pc2[:], rhs=zr[:, :w],
                                 start=True, stop=True)
                q = wp.tile([96, 512], F32, tag="q")
                nc.vector.tensor_scalar(out=q[:, :w], in0=p_ps[:, :w],
                                        scalar1=float(1.0 / (2 * np.pi)),
                                        scalar2=None, op0=ALU.mult)
                ki = wp.tile([96, 512], mybir.dt.int32, tag="ki")
                nc.vector.tensor_copy(ki[:, :w], q[:, :w])
                kf = wp.tile([96, 512], F32, tag="kf")
                nc.vector.tensor_copy(kf[:, :w], ki[:, :w])
                nc.vector.tensor_tensor(out=q[:, :w], in0=q[:, :w], in1=kf[:, :w],
                                        op=ALU.subtract)
                e = wp.tile([96, 512], F32, tag="e")
                nc.scalar.activation(e[:, :w], q[:, :w], AF.Sin, bias=b96[:],
                                     scale=float(2 * np.pi))
                h0p = pq.tile([64, 512], F32, tag="ps")
                nc.tensor.matmul(h0p[:, :w], lhsT=nw[:], rhs=e[:, :w],
                                 start=True, stop=True)
                h0 = wp.tile([64, 512], F32, tag="h0")
                nc.scalar.activation(h0[:, :w], h0p[:, :w], AF.Relu, bias=nb[:])
                h1 = []
                for m in range(2):
                    hp = pq.tile([128, 512], F32, tag="ps")
                    nc.tensor.matmul(hp[:, :w], lhsT=mw1[:, m * 128:(m + 1) * 128],
                                     rhs=h0[:, :w], start=True, stop=True)
                    h = wp.tile([128, 512], F32, tag=f"h1{m}")
                    nc.scalar.activation(h[:, :w], hp[:, :w], AF.Relu,
                                         bias=mb1[:, m:m + 1])
                    h1.append(h)
                h2 = []
                for m in range(2):
                    hp = pq.tile([128, 512], F32, tag="ps")
                    for kc in range(2):
                        nc.tensor.matmul(
                            hp[:, :w],
                            lhsT=mw2[:, kc * 256 + m * 128:kc * 256 + (m + 1) * 128],
                            rhs=h1[kc][:, :w], start=(kc == 0), stop=(kc == 1))
                    h = wp.tile([128, 512], F32, tag=f"h2{m}")
                    nc.scalar.activation(h[:, :w], hp[:, :w], AF.Relu,
                                         bias=mb2[:, m:m + 1])
                    h2.append(h)
                hp = pq.tile([128, 512], F32, tag="ps")
                for kc in range(2):
                    nc.tensor.matmul(hp[:, :w],
                                     lhsT=mw3[:, kc * 128:(kc + 1) * 128],
                                     rhs=h2[kc][:, :w], start=(kc == 0), stop=(kc == 1))
                nc.vector.tensor_scalar(out=dst[:, n0:n0 + w], in0=hp[:, :w],
                                        scalar1=mb3[:, 0:1], scalar2=None,
                                        op0=ALU.add)

            for n0, w in BLKS:
                emb_block(zinv_d, E, n0, w)
            for n0, w in BLKS:
                emb_block(zx0_d, XV, n0, w)

            # ---- emb(sig) -> srhs [128, 2]  (tiny N=2 chain)
            p_ps = pq.tile([96, 2], F32, tag="ps")
            nc.tensor.matmul(p_ps[:], lhsT=pc2[:], rhs=sig[:], start=True, stop=True)
            sq = wp.tile([96, 2], F32, tag="q")
            nc.vector.tensor_scalar(out=sq[:], in0=p_ps[:],
                                    scalar1=float(1.0 / (2 * np.pi)),
                                    scalar2=None, op0=ALU.mult)
            ski = wp.tile([96, 2], mybir.dt.int32, tag="ki")
            nc.vector.tensor_copy(ski[:], sq[:])
            skf = wp.tile([96, 2], F32, tag="kf")
            nc.vector.tensor_copy(skf[:], ski[:])
            nc.vector.tensor_tensor(out=sq[:], in0=sq[:], in1=skf[:], op=ALU.subtract)
            se = wp.tile([96, 2], F32, tag="e")
            nc.scalar.activation(se[:], sq[:], AF.Sin, bias=b96[:],
                                 scale=float(2 * np.pi))
            sh0p = pq.tile([64, 2], F32, tag="ps")
            nc.tensor.matmul(sh0p[:], lhsT=nw[:], rhs=se[:], start=True, stop=True)
            sh0 = wp.tile([64, 2], F32, tag="h0")
            nc.scalar.activation(sh0[:], sh0p[:], AF.Relu, bias=nb[:])
            sh1 = []
            for m in range(2):
                hp = pq.tile([128, 2], F32, tag="ps")
                nc.tensor.matmul(hp[:], lhsT=mw1[:, m * 128:(m + 1) * 128],
                                 rhs=sh0[:], start=True, stop=True)
                h = wp.tile([128, 2], F32, tag=f"h1{m}")
                nc.scalar.activation(h[:], hp[:], AF.Relu, bias=mb1[:, m:m + 1])
                sh1.append(h)
            sh2 = []
            for m in range(2):
                hp = pq.tile([128, 2], F32, tag="ps")
                for kc in range(2):
                    nc.tensor.matmul(
                        hp[:], lhsT=mw2[:, kc * 256 + m * 128:kc * 256 + (m + 1) * 128],
                        rhs=sh1[kc][:], start=(kc == 0), stop=(kc == 1))
                h = wp.tile([128, 2], F32, tag=f"h2{m}")
                nc.scalar.activation(h[:], hp[:], AF.Relu, bias=mb2[:, m:m + 1])
                sh2.append(h)
            hp = pq.tile([128, 2], F32, tag="ps")
            for kc in range(2):
                nc.tensor.matmul(hp[:], lhsT=mw3[:, kc * 128:(kc + 1) * 128],
                                 rhs=sh2[kc][:], start=(kc == 0), stop=(kc == 1))
            srhs = pp.tile([128, 2], F32, tag="srhs")
            nc.vector.tensor_scalar(out=srhs[:], in0=hp[:], scalar1=mb3[:, 0:1],
                                    scalar2=None, op0=ALU.add)

            # ---- two convs, each: global w = sum_n s_v[n]*xv[n,:] via AllReduce
            for conv in range(2):
                wpart = pp.tile([128, 1], F32, tag=f"wpart{conv}")
                nc.vector.memset(wpart[:], 0.0)
                for n0, w in BLKS:
                    zr = wp.tile([1, 512], F32, tag="zr")
                    nc.sync.dma_start(out=zr[:, :w], in_=zsv_d[0:1, n0:n0 + w])
                    bc = pq.tile([128, 512], F32, tag="ps")
                    nc.tensor.matmul(bc[:, :w], lhsT=ones1[:], rhs=zr[:, :w],
                                     start=True, stop=True)
                    nc.vector.tensor_tensor(out=bc[:, :w], in0=XV[:, n0:n0 + w],
                                            in1=bc[:, :w], op=ALU.mult)
                    red = wp.tile([128, 1], F32, tag="red")
                    nc.vector.tensor_reduce(red[:], bc[:, :w],
                                            axis=mybir.AxisListType.X, op=ALU.add)
                    nc.vector.tensor_add(out=wpart[:], in0=wpart[:], in1=red[:])

                arin = dp.tile([128, 1], F32, tag=f"arin{conv}")
                arout = dp.tile([128, 1], F32, tag=f"arout{conv}")
                nc.sync.dma_start(out=arin[:], in_=wpart[:])
                nc.gpsimd.collective_compute(
                    "AllReduce", ALU.add, replica_groups=RG,
                    ins=[arin.opt()], outs=[arout.opt()])
                war = pp.tile([128, 1], F32, tag=f"war{conv}")
                nc.sync.dma_start(out=war[:], in_=arout[:])

                wd = wp.tile([128, 1], F32, tag="wd")
                nc.vector.tensor_scalar(out=wd[:], in0=war[:],
                                        scalar1=1.0 / NUM_EDGES, scalar2=None,
                                        op0=ALU.mult)
                agg = pq.tile([128, 1], F32, tag="ps")
                nc.tensor.matmul(agg[:], lhsT=linw[:, conv * 128:(conv + 1) * 128],
                                 rhs=wd[:], start=True, stop=True)
                rr = pp.tile([128, 1], F32, tag=f"rr{conv}")
                # rr = srhs[:,conv] - (agg + linb[:,conv])
                nc.vector.tensor_tensor(out=rr[:], in0=srhs[:, conv:conv + 1],
                                        in1=agg[:], op=ALU.subtract)
                nc.vector.tensor_tensor(out=rr[:], in0=rr[:],
                                        in1=linb[:, conv:conv + 1], op=ALU.subtract)
                awrr = pp.tile([128, 128], F32, tag=f"awrr{conv}")
                nc.vector.tensor_scalar(out=awrr[:],
                                        in0=actw[:, conv * 128:(conv + 1) * 128],
                                        scalar1=rr[:, 0:1], scalar2=None,
                                        op0=ALU.mult)
                for n0, w in BLKS:
                    ps = pq.tile([128, 512], F32, tag="ps")
                    nc.tensor.matmul(ps[:, :w], lhsT=awrr[:], rhs=E[:, n0:n0 + w],
                                     start=True, stop=False)
                    nc.tensor.matmul(ps[:, :w],
                                     lhsT=actw[:, conv * 128:(conv + 1) * 128],
                                     rhs=XV[:, n0:n0 + w], start=False, stop=True)
                    nc.scalar.activation(XV[:, n0:n0 + w], ps[:, :w], AF.Relu,
                                         bias=actb[:, conv:conv + 1])

            # ---- head
            for n0, w in BLKS:
                p1 = pq.tile([128, 512], F32, tag="ps")
                nc.tensor.matmul(p1[:, :w], lhsT=ow1[:], rhs=XV[:, n0:n0 + w],
                                 start=True, stop=True)
                g1 = wp.tile([128, 512], F32, tag="h10")
                nc.scalar.activation(g1[:, :w], p1[:, :w], AF.Relu, bias=ob1[:])
                p2 = pq.tile([128, 512], F32, tag="ps")
                nc.tensor.matmul(p2[:, :w], lhsT=ow2[:], rhs=g1[:, :w],
                                 start=True, stop=True)
                g2 = wp.tile([128, 512], F32, tag="h11")
                nc.scalar.activation(g2[:, :w], p2[:, :w], AF.Relu, bias=ob2[:])
                p3 = pq.tile([1, 512], F32, tag="ps")
                nc.tensor.matmul(p3[:, :w], lhsT=ow3[:], rhs=g2[:, :w],
                                 start=True, stop=True)
                nc.scalar.activation(out_sb[:, n0:n0 + w], p3[:, :w],
                                     AF.Identity, bias=ob3[:])

            nc.sync.dma_start(out=out_d[:], in_=out_sb[:])
    nc.finalize()
    return nc


# -------------------------------------------------------------------- entry
def kernel(**inputs) -> np.ndarray:
    zinv, zx0, zsv, sig1, sig2 = _host_prep(
        inputs["hyperedge_index"], inputs["coef"], inputs["rhs"])

    pc = np.asarray(inputs["pc"], np.float32).reshape(-1)          # [48]
    vals = {}
    vals["pc2"] = np.concatenate([pc, pc]).reshape(1, 96)
    b96 = np.zeros((96, 1), np.float32); b96[:48] = np.float32(np.pi / 2)
    vals["b96"] = b96
    vals["nw"] = np.asarray(inputs["nw"], np.float32)[0]
    vals["nb"] = np.asarray(inputs["nb"], np.float32).reshape(64, 1)
    vals["mw1"] = np.asarray(inputs["mw1"], np.float32)
    vals["mb1"] = np.asarray(inputs["mb1"], np.float32).reshape(2, 128).T.copy()
    mw2 = np.asarray(inputs["mw2"], np.float32)
    vals["mw2"] = np.concatenate([mw2[:128], mw2[128:]], axis=1)
    vals["mb2"] = np.asarray(inputs["mb2"], np.float32).reshape(2, 128).T.copy()
    mw3 = np.asarray(inputs["mw3"], np.float32)
    vals["mw3"] = np.concatenate([mw3[:128], mw3[128:]], axis=1)
    vals["mb3"] = np.asarray(inputs["mb3"], np.float32).reshape(128, 1)
    linw = np.asarray(inputs["lin_c_w"], np.float32)
    vals["linw"] = np.concatenate([linw[0], linw[1]], axis=1)
    vals["linb"] = np.asarray(inputs["lin_c_b"], np.float32).T.copy()
    actw = np.asarray(inputs["act_w"], np.float32)
    vals["actw"] = np.concatenate([actw[0], actw[1]], axis=1)
    vals["actb"] = np.asarray(inputs["act_b"], np.float32).T.copy()
    vals["ow1"] = np.asarray(inputs["ow1"], np.float32)
    vals["ob1"] = np.asarray(inputs["ob1"], np.float32).reshape(128, 1)
    vals["ow2"] = np.asarray(inputs["ow2"], np.float32)
    vals["ob2"] = np.asarray(inputs["ob2"], np.float32).reshape(128, 1)
    vals["ow3"] = np.asarray(inputs["ow3"], np.float32).reshape(128, 1)
    vals["ob3"] = np.asarray(inputs["ob3"], np.float32).reshape(1, 1)
    vals["sig"] = np.array([[sig1, sig2]], np.float32) * np.float32(2 * np.pi)

    wpack = np.zeros((128, WCOLS), np.float32)
    for name, (r, c, o) in WSPEC.items():
        wpack[0:r, o:o + c] = vals[name]

    shared = dict(wpk=wpack)
    in_maps = [dict(shared,
                    zinv=np.ascontiguousarray(
                        (np.float32(2 * np.pi) * zinv[p]).reshape(1, NSH)),
                    zx0=np.ascontiguousarray(
                        (np.float32(2 * np.pi) * zx0[p]).reshape(1, NSH)),
                    zsv=np.ascontiguousarray(zsv[p].reshape(1, NSH)))
               for p in range(NCORES)]

    _CACHE["in_maps"] = in_maps
    if "nc" not in _CACHE:
        _CACHE["nc"] = _build_nc()
    res = run_bass_kernel_spmd(_CACHE["nc"], in_maps, core_ids=list(range(NCORES)))
    outs = [res.results[p]["out"].reshape(-1) for p in range(NCORES)]
    full = np.concatenate(outs)[:NUM_NODES].astype(np.float32)
    return full.reshape(NUM_NODES, 1)



# revision 6
# speedup vs baseline: 1.2551x; 1.2551x over previous
"""Bass/Trainium2 kernel for nn_GNNPolicy_MILP (gnn_message_passing).

Strategy (8 NeuronCores, SPMD):
  - Host: cheap scalar graph prep on the nnz/constraint axis (segment sums via
    bincount, ~1.5% of total FLOPs) producing per-node z-inputs inv_s_v, x0.
    Nodes padded 100000 -> 100352 = 8*12544 and row-sharded per core.
  - Device (per core, fp32 compute): one fused 128-wide embedding MLP pass over
    2*12544 z-values (both tables at once, feature-major), two conv updates with
    a [128] AllReduce each, 3-layer output head.

Wire-size optimizations (the dispatch wall-clock is transfer-dominated):
  - Weights packed fp16 [128,1870], row-sharded [16,1870] per core and
    AllGathered on device, then upcast to fp32 in SBUF (rel err ~8e-4).
  - emb(sum_rhs) for both convs computed on host (2 scalars) -> tiny input.
  - z-inputs packed as one [1, 3*NSH] row per core: x0 | inv_s_v | s_v.
  - Output fp16.

Exact algebraic reductions vs the reference (not approximations):
  - emb_rhs is dead code; s_c/scaled_coef/s_v are identical across both convs.
  - mean(x_const) == (1/E) * sum_n s_v[n] * x_var[n]  -- the [50k,128]
    gather/scatter collapses to a weighted reduction over nodes.
"""
import numpy as np

import concourse.bass as bass
from concourse import bacc
import concourse.mybir as mybir
import concourse.tile as tile
from concourse.bass_utils import run_bass_kernel_spmd

NUM_NODES = 100000
NUM_EDGES = 50000
HID = 128
NCORES = 8
NSH = 12544            # padded nodes per core (8*12544 = 100352)
NZ2 = 2 * NSH          # emb inputs: x0|inv
NZ3 = 3 * NSH          # + s_v row
F32 = mybir.dt.float32
F16 = mybir.dt.float16
I32 = mybir.dt.int32
TWO_PI = float(2.0 * np.pi)

_CACHE = {}

# fp16 weight pack: name -> (rows, cols); column-packed into [128, WCOLS]
_WLIST = [("nw", 96, 64), ("nb", 64, 1),
          ("mw1", 64, 256), ("mb1", 128, 2), ("mw2", 128, 512), ("mb2", 128, 2),
          ("mw3", 128, 256), ("mb3", 128, 1), ("linw", 128, 256), ("linb", 128, 2),
          ("actw", 128, 256), ("actb", 128, 2), ("ow1", 128, 128), ("ob1", 128, 1),
          ("ow2", 128, 128), ("ob2", 128, 1), ("ow3", 128, 1), ("ob3", 1, 1)]
WSPEC = {}
_o = 0
for _n, _r, _c in _WLIST:
    WSPEC[_n] = (_r, _c, _o)
    _o += _c
WCOLS = _o  # 1870
assert WCOLS % 2 == 0

BLK = 2048
EMB_BLKS = [(b * BLK, BLK) for b in range(NZ2 // BLK)] + (
    [(NZ2 - NZ2 % BLK, NZ2 % BLK)] if NZ2 % BLK else [])
X_BLKS = [(b * BLK, BLK) for b in range(NSH // BLK)] + (
    [(NSH - NSH % BLK, NSH % BLK)] if NSH % BLK else [])


# --------------------------------------------------------------------- host
def _emb_host(z, pc, nw, nb, mw1, mb1, mw2, mb2, mw3, mb3):
    """float64 replica of reference _rhs_emb for one scalar z -> [HID]."""
    p = 2.0 * np.pi * np.float64(z) * pc.astype(np.float64).reshape(-1)
    e = np.concatenate([np.cos(p), np.sin(p)])
    h = np.maximum(e @ nw[0].astype(np.float64) + nb.astype(np.float64).reshape(-1), 0.0)
    h = np.maximum(h @ mw1.astype(np.float64) + mb1.astype(np.float64), 0.0)
    h = np.maximum(h @ mw2.astype(np.float64) + mb2.astype(np.float64), 0.0)
    return h @ mw3.astype(np.float64) + mb3.astype(np.float64)


def _host_prep(hyperedge_index, coef, rhs):
    row = np.asarray(hyperedge_index[0]).astype(np.int64)
    col = np.asarray(hyperedge_index[1]).astype(np.int64)
    coef = np.asarray(coef, np.float64)
    rhs = np.asarray(rhs, np.float64).reshape(-1)

    colc = np.minimum(col, NUM_EDGES - 1)  # mimic jax clamp for oob ids
    s_c = np.bincount(colc, weights=np.abs(coef), minlength=NUM_EDGES)[:NUM_EDGES]
    inv_s_c = np.where(s_c == 0, 0.0, 1.0 / np.where(s_c == 0, 1.0, s_c))
    sc = (coef * inv_s_c[colc]).astype(np.float64)
    rhs1 = rhs * inv_s_c
    rhs2 = rhs1 * inv_s_c
    sig1 = np.float32(rhs1.sum())
    sig2 = np.float32(rhs2.sum())

    s_v = np.bincount(row, weights=sc, minlength=NUM_NODES)[:NUM_NODES]
    x0pre = np.bincount(row, weights=sc * rhs1[colc], minlength=NUM_NODES)[:NUM_NODES]
    inv_s_v = np.where(s_v == 0, 0.0, 1.0 / np.where(s_v == 0, 1.0, s_v)).astype(np.float32)
    x0 = (inv_s_v * x0pre).astype(np.float32)

    def shard(a):
        p = np.zeros(NCORES * NSH, np.float32)
        p[:NUM_NODES] = a
        return p.reshape(NCORES, NSH)

    return shard(inv_s_v), shard(x0), shard(s_v.astype(np.float32)), sig1, sig2


# ------------------------------------------------------------------- device
def _build_nc():
    nc = bacc.Bacc(None, num_devices=NCORES)

    wsh_d = nc.dram_tensor("wsh", [128 // NCORES, WCOLS], F16, kind="ExternalInput")
    z2_d = nc.dram_tensor("z2", [1, NZ3], F32, kind="ExternalInput")
    pc_d = nc.dram_tensor("pcrow", [1, 96], F32, kind="ExternalInput")
    sr_d = nc.dram_tensor("srhs", [128, 2], F32, kind="ExternalInput")
    out_d = nc.dram_tensor("out", [1, NSH], F16, kind="ExternalOutput")

    AF = mybir.ActivationFunctionType
    ALU = mybir.AluOpType
    RG = [list(range(NCORES))]

    with tile.TileContext(nc) as tc:
        with (
            tc.tile_pool(name="persist", bufs=1) as pp,
            tc.tile_pool(name="work", bufs=1) as wp,
            tc.tile_pool(name="psum", bufs=2, space="PSUM") as pq,
            tc.tile_pool(name="dram", bufs=1, space="DRAM") as dp,
        ):
            # ---- weights: AllGather fp16 shards, upcast to fp32
            wstage = dp.tile([128 // NCORES, WCOLS], F16, tag="wstage")
            nc.sync.dma_start(out=wstage[:], in_=wsh_d[:])
            wg = dp.tile([128, WCOLS], F16, tag="wg")
            nc.gpsimd.collective_compute(
                "AllGather", ALU.bypass, replica_groups=RG,
                ins=[wstage.opt()], outs=[wg.opt()])
            w16 = wp.tile([128, WCOLS], F16, tag="w16")
            nc.sync.dma_start(out=w16[:], in_=wg[:])
            wpk = pp.tile([128, WCOLS], F32, tag="wpk")
            nc.vector.tensor_copy(wpk[:], w16[:])

            def wsl(name):
                r, c, o = WSPEC[name]
                return wpk[0:r, o:o + c]

            nw = wsl("nw"); nb = wsl("nb")
            mw1 = wsl("mw1"); mb1 = wsl("mb1")
            mw2 = wsl("mw2"); mb2 = wsl("mb2")
            mw3 = wsl("mw3"); mb3 = wsl("mb3")
            linw = wsl("linw"); linb = wsl("linb")
            actw = wsl("actw"); actb = wsl("actb")
            ow1 = wsl("ow1"); ob1 = wsl("ob1")
            ow2 = wsl("ow2"); ob2 = wsl("ob2")
            ow3 = wsl("ow3"); ob3 = wsl("ob3")

            pcr = pp.tile([1, 96], F32, tag="pcr")
            nc.sync.dma_start(out=pcr[:], in_=pc_d[:])
            srhs = pp.tile([128, 2], F32, tag="srhs")
            nc.sync.dma_start(out=srhs[:], in_=sr_d[:])
            b96 = pp.tile([96, 1], F32, tag="b96")
            nc.vector.memset(b96[:], 0.0)
            nc.vector.memset(b96[0:48, :], float(np.pi / 2))
            ones1 = pp.tile([1, 128], F32, tag="ones1")
            nc.vector.memset(ones1[:], 1.0)

            T = pp.tile([128, NZ2], F32, tag="T")     # [XV | E]

            # ---- fused emb over both tables: T[:, i] = emb(z2[i])
            for n0, w in EMB_BLKS:
                zr = wp.tile([1, BLK], F32, tag="zr")
                nc.sync.dma_start(out=zr[:, :w], in_=z2_d[0:1, n0:n0 + w])
                psA = pq.tile([128, BLK], F32, tag="ps")
                for j in range(0, w, 512):
                    jw = min(512, w - j)
                    nc.tensor.matmul(psA[0:96, j:j + jw], lhsT=pcr[:],
                                     rhs=zr[:, j:j + jw], start=True, stop=True)
                ki = wp.tile([96, BLK], I32, tag="ki")
                nc.vector.tensor_copy(ki[:, :w], psA[0:96, :w])
                kf = wp.tile([96, BLK], F32, tag="kf")
                nc.vector.tensor_copy(kf[:, :w], ki[:, :w])
                q = wp.tile([96, BLK], F32, tag="q")
                nc.vector.tensor_tensor(out=q[:, :w], in0=psA[0:96, :w],
                                        in1=kf[:, :w], op=ALU.subtract)
                e = wp.tile([96, BLK], F32, tag="e")
                nc.scalar.activation(e[:, :w], q[:, :w], AF.Sin, bias=b96[:],
                                     scale=TWO_PI)
                psB = pq.tile([128, BLK], F32, tag="ps")
                for j in range(0, w, 512):
                    jw = min(512, w - j)
                    nc.tensor.matmul(psB[0:64, j:j + jw], lhsT=nw[:],
                                     rhs=e[:, j:j + jw], start=True, stop=True)
                h0 = wp.tile([64, BLK], F32, tag="h0")
                nc.scalar.activation(h0[:, :w], psB[0:64, :w], AF.Relu, bias=nb[:])
                h1 = []
                for m in range(2):
                    hp = pq.tile([128, BLK], F32, tag="ps")
                    for j in range(0, w, 512):
                        jw = min(512, w - j)
                        nc.tensor.matmul(hp[:, j:j + jw],
                                         lhsT=mw1[:, m * 128:(m + 1) * 128],
                                         rhs=h0[:, j:j + jw], start=True, stop=True)
                    h = wp.tile([128, BLK], F32, tag=f"h1{m}")
                    nc.scalar.activation(h[:, :w], hp[:, :w], AF.Relu,
                                         bias=mb1[:, m:m + 1])
                    h1.append(h)
                h2 = []
                for m in range(2):
                    hp = pq.tile([128, BLK], F32, tag="ps")
                    for j in range(0, w, 512):
                        jw = min(512, w - j)
                        for kc in range(2):
                            nc.tensor.matmul(
                                hp[:, j:j + jw],
                                lhsT=mw2[:, kc * 256 + m * 128:kc * 256 + (m + 1) * 128],
                                rhs=h1[kc][:, j:j + jw],
                                start=(kc == 0), stop=(kc == 1))
                    h = wp.tile([128, BLK], F32, tag=f"h2{m}")
                    nc.scalar.activation(h[:, :w], hp[:, :w], AF.Relu,
                                         bias=mb2[:, m:m + 1])
                    h2.append(h)
                hp = pq.tile([128, BLK], F32, tag="ps")
                for j in range(0, w, 512):
                    jw = min(512, w - j)
                    for kc in range(2):
                        nc.tensor.matmul(hp[:, j:j + jw],
                                         lhsT=mw3[:, kc * 128:(kc + 1) * 128],
                                         rhs=h2[kc][:, j:j + jw],
                                         start=(kc == 0), stop=(kc == 1))
                nc.vector.tensor_scalar(out=T[:, n0:n0 + w], in0=hp[:, :w],
                                        scalar1=mb3[:, 0:1], scalar2=None,
                                        op0=ALU.add)

            # ---- two convs: global w = sum_n s_v[n]*xv[n,:] via AllReduce
            for conv in range(2):
                wpart = pp.tile([128, 1], F32, tag=f"wpart{conv}")
                nc.vector.memset(wpart[:], 0.0)
                for n0, w in X_BLKS:
                    sv = wp.tile([1, BLK], F32, tag="sv")
                    nc.sync.dma_start(out=sv[:, :w],
                                      in_=z2_d[0:1, NZ2 + n0:NZ2 + n0 + w])
                    bc = pq.tile([128, BLK], F32, tag="ps")
                    for j in range(0, w, 512):
                        jw = min(512, w - j)
                        nc.tensor.matmul(bc[:, j:j + jw], lhsT=ones1[:],
                                         rhs=sv[0:1, j:j + jw],
                                         start=True, stop=True)
                    nc.vector.tensor_tensor(out=bc[:, :w], in0=T[:, n0:n0 + w],
                                            in1=bc[:, :w], op=ALU.mult)
                    red = wp.tile([128, 1], F32, tag="red")
                    nc.vector.tensor_reduce(red[:], bc[:, :w],
                                            axis=mybir.AxisListType.X, op=ALU.add)
                    nc.vector.tensor_add(out=wpart[:], in0=wpart[:], in1=red[:])

                arin = dp.tile([128, 1], F32, tag=f"arin{conv}")
                arout = dp.tile([128, 1], F32, tag=f"arout{conv}")
                nc.sync.dma_start(out=arin[:], in_=wpart[:])
                nc.gpsimd.collective_compute(
                    "AllReduce", ALU.add, replica_groups=RG,
                    ins=[arin.opt()], outs=[arout.opt()])
                war = wp.tile([128, 1], F32, tag="war")
                nc.sync.dma_start(out=war[:], in_=arout[:])

                wd = wp.tile([128, 1], F32, tag="wd")
                nc.vector.tensor_scalar(out=wd[:], in0=war[:],
                                        scalar1=float(1.0 / NUM_EDGES),
                                        scalar2=None, op0=ALU.mult)
                agg = pq.tile([128, BLK], F32, tag="ps")
                nc.tensor.matmul(agg[:, 0:1], lhsT=linw[:, conv * 128:(conv + 1) * 128],
                                 rhs=wd[:], start=True, stop=True)
                rr = wp.tile([128, 1], F32, tag="rr")
                nc.vector.tensor_tensor(out=rr[:], in0=srhs[:, conv:conv + 1],
                                        in1=agg[:, 0:1], op=ALU.subtract)
                nc.vector.tensor_tensor(out=rr[:], in0=rr[:],
                                        in1=linb[:, conv:conv + 1], op=ALU.subtract)
                awrr = wp.tile([128, 128], F32, tag="awrr")
                nc.vector.tensor_scalar(out=awrr[:],
                                        in0=actw[:, conv * 128:(conv + 1) * 128],
                                        scalar1=rr[:, 0:1], scalar2=None,
                                        op0=ALU.mult)
                for n0, w in X_BLKS:
                    ps = pq.tile([128, BLK], F32, tag="ps")
                    for j in range(0, w, 512):
                        jw = min(512, w - j)
                        nc.tensor.matmul(ps[:, j:j + jw], lhsT=awrr[:],
                                         rhs=T[:, NSH + n0 + j:NSH + n0 + j + jw],
                                         start=True, stop=False)
                        nc.tensor.matmul(ps[:, j:j + jw],
                                         lhsT=actw[:, conv * 128:(conv + 1) * 128],
                                         rhs=T[:, n0 + j:n0 + j + jw],
                                         start=False, stop=True)
                    nc.scalar.activation(T[:, n0:n0 + w], ps[:, :w], AF.Relu,
                                         bias=actb[:, conv:conv + 1])

            # ---- head
            for n0, w in X_BLKS:
                p1 = pq.tile([128, BLK], F32, tag="ps")
                for j in range(0, w, 512):
                    jw = min(512, w - j)
                    nc.tensor.matmul(p1[:, j:j + jw], lhsT=ow1[:],
                                     rhs=T[:, n0 + j:n0 + j + jw],
                                     start=True, stop=True)
                g1 = wp.tile([128, BLK], F32, tag="h10")
                nc.scalar.activation(g1[:, :w], p1[:, :w], AF.Relu, bias=ob1[:])
                p2 = pq.tile([128, BLK], F32, tag="ps")
                for j in range(0, w, 512):
                    jw = min(512, w - j)
                    nc.tensor.matmul(p2[:, j:j + jw], lhsT=ow2[:],
                                     rhs=g1[:, j:j + jw], start=True, stop=True)
                g2 = wp.tile([128, BLK], F32, tag="h11")
                nc.scalar.activation(g2[:, :w], p2[:, :w], AF.Relu, bias=ob2[:])
                p3 = pq.tile([128, BLK], F32, tag="ps")
                for j in range(0, w, 512):
                    jw = min(512, w - j)
                    nc.tensor.matmul(p3[0:1, j:j + jw], lhsT=ow3[:],
                                     rhs=g2[:, j:j + jw], start=True, stop=True)
                ob = wp.tile([1, BLK], F16, tag="ob")
                nc.scalar.activation(ob[:, :w], p3[0:1, :w],
                                     AF.Identity, bias=ob3[:])
                nc.sync.dma_start(out=out_d[0:1, n0:n0 + w], in_=ob[:, :w])
    nc.finalize()
    return nc


# -------------------------------------------------------------------- entry
def kernel(**inputs) -> np.ndarray:
    zinv, zx0, zsv, sig1, sig2 = _host_prep(
        inputs["hyperedge_index"], inputs["coef"], inputs["rhs"])

    pc = np.asarray(inputs["pc"], np.float32).reshape(-1)          # [48]
    pcrow = np.concatenate([pc, pc]).reshape(1, 96).astype(np.float32)

    vals = {}
    vals["nw"] = np.asarray(inputs["nw"], np.float32)[0]
    vals["nb"] = np.asarray(inputs["nb"], np.float32).reshape(64, 1)
    vals["mw1"] = np.asarray(inputs["mw1"], np.float32)
    vals["mb1"] = np.asarray(inputs["mb1"], np.float32).reshape(2, 128).T.copy()
    mw2 = np.asarray(inputs["mw2"], np.float32)
    vals["mw2"] = np.concatenate([mw2[:128], mw2[128:]], axis=1)
    vals["mb2"] = np.asarray(inputs["mb2"], np.float32).reshape(2, 128).T.copy()
    mw3 = np.asarray(inputs["mw3"], np.float32)
    vals["mw3"] = np.concatenate([mw3[:128], mw3[128:]], axis=1)
    vals["mb3"] = np.asarray(inputs["mb3"], np.float32).reshape(128, 1)
    linw = np.asarray(inputs["lin_c_w"], np.float32)
    vals["linw"] = np.concatenate([linw[0], linw[1]], axis=1)
    vals["linb"] = np.asarray(inputs["lin_c_b"], np.float32).T.copy()
    actw = np.asarray(inputs["act_w"], np.float32)
    vals["actw"] = np.concatenate([actw[0], actw[1]], axis=1)
    vals["actb"] = np.asarray(inputs["act_b"], np.float32).T.copy()
    vals["ow1"] = np.asarray(inputs["ow1"], np.float32)
    vals["ob1"] = np.asarray(inputs["ob1"], np.float32).reshape(128, 1)
    vals["ow2"] = np.asarray(inputs["ow2"], np.float32)
    vals["ob2"] = np.asarray(inputs["ob2"], np.float32).reshape(128, 1)
    vals["ow3"] = np.asarray(inputs["ow3"], np.float32).reshape(128, 1)
    vals["ob3"] = np.asarray(inputs["ob3"], np.float32).reshape(1, 1)

    wpack = np.zeros((128, WCOLS), np.float16)
    for name, (r, c, o) in WSPEC.items():
        wpack[0:r, o:o + c] = vals[name].astype(np.float16)
    wshards = wpack.reshape(NCORES, 128 // NCORES, WCOLS)

    # emb(sum_rhs) for both convs on host (2 scalars)
    ekw = dict(pc=np.asarray(inputs["pc"], np.float32),
               nw=np.asarray(inputs["nw"], np.float32),
               nb=np.asarray(inputs["nb"], np.float32),
               mw1=np.asarray(inputs["mw1"], np.float32),
               mb1=np.asarray(inputs["mb1"], np.float32),
               mw2=np.asarray(inputs["mw2"], np.float32),
               mb2=np.asarray(inputs["mb2"], np.float32),
               mw3=np.asarray(inputs["mw3"], np.float32),
               mb3=np.asarray(inputs["mb3"], np.float32))
    srhs = np.stack([_emb_host(sig1, **ekw), _emb_host(sig2, **ekw)],
                    axis=1).astype(np.float32)                     # [128, 2]

    in_maps = []
    for p in range(NCORES):
        z2 = np.empty((1, NZ3), np.float32)
        z2[0, :NSH] = zx0[p]
        z2[0, NSH:NZ2] = zinv[p]
        z2[0, NZ2:] = zsv[p]
        in_maps.append(dict(wsh=np.ascontiguousarray(wshards[p]),
                            z2=z2, pcrow=pcrow, srhs=srhs))

    _CACHE["in_maps"] = in_maps
    if "nc" not in _CACHE:
        _CACHE["nc"] = _build_nc()
    res = run_bass_kernel_spmd(_CACHE["nc"], in_maps, core_ids=list(range(NCORES)))
    outs = [res.results[p]["out"].reshape(-1) for p in range(NCORES)]
    full = np.concatenate(outs)[:NUM_NODES].astype(np.float32)
    return full.reshape(NUM_NODES, 1)
